# revision 11
# baseline (speedup 1.0000x reference)
"""Trainium2 Bass kernel for MoE feed-forward (nn_MoEFeedForward_12292196401617).

Reference computation (per batch b of 32, N=1024 tokens, DIM=1024):
    h      = gelu_erf(x @ fc1_w.T + fc1_b)                  # [B,N,HID=4096]
    shared = h @ fc2_w.T + fc2_b                            # [B,N,768]
    expert = h @ experts_w[idx[b]].T + experts_b[idx[b]]    # [B,N,256]
    out    = concat([shared, expert], -1)                   # [B,N,1024]

Strategy: data-parallel over batch across 8 NeuronCores (4 batches/core).
The expert gather is resolved on the host (indices are per-batch), so the
device program is pure dense matmul. Everything is laid out
feature-on-partitions / tokens-on-free-dim, so all host->device tensors are
pre-transposed on the host. Weights are cast to bf16 (PE runs bf16 at 1
cycle/row vs 4 for fp32) and stay resident in SBUF; accumulation in fp32
PSUM; gelu runs on the scalar engine (erf-exact Gelu) fused with the fc1
bias add; the fc2/expert bias add is fused into the PSUM->SBUF eviction.
"""

import sys

sys.path.insert(0, "/opt/trn_rl_repo")

import numpy as np
import ml_dtypes

B, N, DIM = 32, 1024, 1024
HID = 4096
PART = 256
OUT = 1024
SHARED = OUT - PART  # 768
E = 16

NCORES = 8
BPC = B // NCORES        # batches per core = 4
TOK = BPC * N            # tokens per core  = 4096
TT = 512                 # token tile
NTILES = TOK // TT       # 8  (2 tiles per batch)
P = 128
KC = DIM // P            # 8  fc1 contraction chunks
HC = HID // P            # 32 hidden chunks
OC = OUT // P            # 8  output chunks (6 shared + 2 expert)
SC = SHARED // P         # 6

_CACHE: dict = {}


def _build_program():
    import concourse.tile as tile
    from concourse import bacc, mybir

    bf16 = mybir.dt.bfloat16
    f32 = mybir.dt.float32
    GELU = mybir.ActivationFunctionType.Gelu
    IDENT = mybir.ActivationFunctionType.Identity

    nc = bacc.Bacc()
    xT_d = nc.declare_dram_parameter("xT", [DIM, TOK], bf16, isOutput=False)
    w1T_d = nc.declare_dram_parameter("w1T", [DIM, HID], bf16, isOutput=False)
    b1T_d = nc.declare_dram_parameter("b1T", [P, HC], f32, isOutput=False)
    w2T_d = nc.declare_dram_parameter("w2T", [HID, SHARED], bf16, isOutput=False)
    weT_d = nc.declare_dram_parameter("weT", [BPC, HID, PART], bf16, isOutput=False)
    b2T_d = nc.declare_dram_parameter("b2T", [P, BPC * OC], f32, isOutput=False)
    outT_d = nc.declare_dram_parameter("outT", [OUT, TOK], f32, isOutput=True)

    with tile.TileContext(nc) as tc:
        with (
            tc.tile_pool(name="wsb", bufs=1) as wsb,      # resident weights
            tc.tile_pool(name="wesb", bufs=1) as wesb,    # expert weights (per batch)
            tc.tile_pool(name="bsb", bufs=1) as bsb,      # biases
            tc.tile_pool(name="xsb", bufs=2) as xsb,      # x tiles, double buffered
            tc.tile_pool(name="hsb", bufs=1) as hsb,      # gelu output chunks
            tc.tile_pool(name="osb", bufs=4) as osb,      # out staging
            tc.tile_pool(name="hps", bufs=2, space="PSUM") as hps,
            tc.tile_pool(name="ops", bufs=6, space="PSUM") as ops,
        ):
            # ---- load order matters: the first fc1 chain needs x tile 0 +
            # w1 only; w2 / expert weights aren't read until the PE is ~55us
            # in, so they load behind the critical path.
            b1_t = bsb.tile([P, HC], f32, tag="b1")
            nc.gpsimd.dma_start(b1_t[:], b1T_d[:, :])
            b2_t = bsb.tile([P, BPC * OC], f32, tag="b2")
            nc.gpsimd.dma_start(b2_t[:], b2T_d[:, :])

            # PE warmup: dummy matmuls on an uninitialized scratch tile keep
            # the PE busy from the preamble until the first x/w1 bytes land,
            # so the HAM clock gate starts ramping toward 8/8 immediately.
            # The results are never read.
            scr = bsb.tile([P, TT], bf16, tag="scr")
            nc.vector.memset(scr[:], 0.0)
            for _ in range(20):
                wp = hps.tile([P, TT], f32, tag="hps", name="warm")
                nc.tensor.matmul(wp[:], scr[:, 0:P], scr[:], start=True, stop=True)

            def load_x(ti, engine=None):
                t0 = ti * TT
                tiles = []
                for kc in range(KC):
                    t = xsb.tile([P, TT], bf16, tag=f"x_{kc}")
                    (engine or nc.sync).dma_start(
                        t[:], xT_d[kc * P:(kc + 1) * P, t0:t0 + TT]
                    )
                    tiles.append(t)
                return tiles

            def load_we(b, we_t):
                for hc in range(HC):
                    t = wesb.tile([P, PART], bf16, tag=f"we_{hc}")
                    nc.sync.dma_start(t[:], weT_d[b, hc * P:(hc + 1) * P, :])
                    we_t[hc] = t

            # x tile 0 goes over the SWDGE (gpsimd) path so it streams in
            # parallel with the HWDGE w1 loads below.
            x_pending = load_x(0, engine=nc.gpsimd)

            # w1 loads in 512-wide column slices, earliest-needed first: the
            # fc1 chain for hid chunk hc only reads w1 columns
            # [hc*128,(hc+1)*128), so the PE can start once slice 0 (hid
            # chunks 0..3) has landed.
            w1_t = [
                wsb.tile([P, HID], bf16, tag=f"w1_{kc}", name=f"w1_{kc}")
                for kc in range(KC)
            ]
            HS = 512
            for q in range(HID // HS):
                for kc in range(KC):
                    nc.sync.dma_start(
                        w1_t[kc][:, q * HS:(q + 1) * HS],
                        w1T_d[kc * P:(kc + 1) * P, q * HS:(q + 1) * HS],
                    )

            w2_t = []
            for hc in range(HC):
                t = wsb.tile([P, SHARED], bf16, tag=f"w2_{hc}")
                nc.sync.dma_start(t[:], w2T_d[hc * P:(hc + 1) * P, :])
                w2_t.append(t)

            we_t = [None] * HC  # current batch's expert weight chunks
            load_we(0, we_t)

            for ti in range(NTILES):
                b = ti // (NTILES // BPC)
                t0 = ti * TT
                if ti % (NTILES // BPC) == 0 and ti > 0:
                    load_we(b, we_t)

                x_t = x_pending
                if ti + 1 < NTILES:
                    x_pending = load_x(ti + 1)

                # fc1 + erf-gelu: h^T[hid, tok] per 128-row chunk
                h_t = []
                for hc in range(HC):
                    acc = hps.tile([P, TT], f32, tag="hps")
                    for kc in range(KC):
                        nc.tensor.matmul(
                            acc[:],
                            w1_t[kc][:, hc * P:(hc + 1) * P],
                            x_t[kc][:],
                            start=(kc == 0),
                            stop=(kc == KC - 1),
                        )
                    h = hsb.tile([P, TT], bf16, tag=f"h_{hc}")
                    nc.scalar.activation(
                        h[:], acc[:], GELU, bias=b1_t[:, hc:hc + 1], scale=1.0
                    )
                    h_t.append(h)

                # fc2 (shared) + expert projection: out^T[out, tok]
                for oc in range(OC):
                    acc = ops.tile([P, TT], f32, tag="ops")
                    for hc in range(HC):
                        if oc < SC:
                            w = w2_t[hc][:, oc * P:(oc + 1) * P]
                        else:
                            w = we_t[hc][:, (oc - SC) * P:(oc - SC + 1) * P]
                        nc.tensor.matmul(
                            acc[:], w, h_t[hc][:],
                            start=(hc == 0), stop=(hc == HC - 1),
                        )
                    o = osb.tile([P, TT], f32, tag="o")
                    nc.scalar.activation(
                        o[:], acc[:], IDENT,
                        bias=b2_t[:, b * OC + oc:b * OC + oc + 1], scale=1.0,
                    )
                    nc.sync.dma_start(outT_d[oc * P:(oc + 1) * P, t0:t0 + TT], o[:])

    nc.finalize()
    return nc


def _get_program():
    if "nc" not in _CACHE:
        _CACHE["nc"] = _build_program()
    return _CACHE["nc"]


def _prep_in_maps(x, indices, fc1_w, fc1_b, fc2_w, fc2_b, experts_w, experts_b):
    bf16 = ml_dtypes.bfloat16
    x = np.asarray(x, dtype=np.float32)
    indices = np.asarray(indices).astype(np.int64)
    fc1_w = np.asarray(fc1_w, dtype=np.float32)
    fc1_b = np.asarray(fc1_b, dtype=np.float32)
    fc2_w = np.asarray(fc2_w, dtype=np.float32)
    fc2_b = np.asarray(fc2_b, dtype=np.float32)
    experts_w = np.asarray(experts_w, dtype=np.float32)
    experts_b = np.asarray(experts_b, dtype=np.float32)

    w1T = fc1_w.T.astype(bf16)                       # [DIM, HID]
    b1T = np.ascontiguousarray(fc1_b.reshape(HC, P).T)   # [P, HC]
    w2T = fc2_w.T.astype(bf16)                       # [HID, SHARED]

    in_maps = []
    for c in range(NCORES):
        idx = indices[c * BPC:(c + 1) * BPC]         # [BPC]
        xs = x[c * BPC:(c + 1) * BPC]                # [BPC, N, DIM]
        xT = xs.reshape(TOK, DIM).T.astype(bf16)     # [DIM, TOK]
        weT = experts_w[idx].transpose(0, 2, 1).astype(bf16)  # [BPC, HID, PART]
        b2 = np.concatenate(
            [np.broadcast_to(fc2_b, (BPC, SHARED)), experts_b[idx]], axis=1
        )                                            # [BPC, OUT]
        b2T = np.ascontiguousarray(
            b2.reshape(BPC, OC, P).transpose(2, 0, 1).reshape(P, BPC * OC)
        ).astype(np.float32)                         # [P, BPC*OC]
        in_maps.append({
            "xT": xT, "w1T": w1T, "b1T": b1T, "w2T": w2T,
            "weT": weT, "b2T": b2T,
        })
    return in_maps


def _assemble_output(results):
    out = np.empty((B, N, OUT), dtype=np.float32)
    for c in range(NCORES):
        outT = results[c]["outT"]                    # [OUT, TOK]
        out[c * BPC:(c + 1) * BPC] = outT.T.reshape(BPC, N, OUT)
    return out


def run_on_device(inputs: dict, trace: bool = False):
    """Run the SPMD program; returns (full_output, BassKernelResults)."""
    from concourse.bass_utils import run_bass_kernel_spmd

    nc = _get_program()
    in_maps = _prep_in_maps(**inputs)
    res = run_bass_kernel_spmd(nc, in_maps, list(range(NCORES)), trace=trace)
    return _assemble_output(res.results), res


def kernel(**inputs) -> np.ndarray:
    out, _ = run_on_device(inputs, trace=False)
    return out


# revision 12
# speedup vs baseline: 1.0021x; 1.0021x over previous
"""Trainium2 Bass kernel for MoE feed-forward (nn_MoEFeedForward_12292196401617).

Reference computation (per batch b of 32, N=1024 tokens, DIM=1024):
    h      = gelu_erf(x @ fc1_w.T + fc1_b)                  # [B,N,HID=4096]
    shared = h @ fc2_w.T + fc2_b                            # [B,N,768]
    expert = h @ experts_w[idx[b]].T + experts_b[idx[b]]    # [B,N,256]
    out    = concat([shared, expert], -1)                   # [B,N,1024]

Strategy: data-parallel over batch across 8 NeuronCores (4 batches/core).
The expert gather is resolved on the host (indices are per-batch), so the
device program is pure dense matmul. Everything is laid out
feature-on-partitions / tokens-on-free-dim, so all host->device tensors are
pre-transposed on the host. Weights are cast to bf16 (PE runs bf16 at 1
cycle/row vs 4 for fp32) and stay resident in SBUF; accumulation in fp32
PSUM; gelu runs on the scalar engine (erf-exact Gelu) fused with the fc1
bias add; the fc2/expert bias add is fused into the PSUM->SBUF eviction.
"""

import sys

sys.path.insert(0, "/opt/trn_rl_repo")

import numpy as np
import ml_dtypes

B, N, DIM = 32, 1024, 1024
HID = 4096
PART = 256
OUT = 1024
SHARED = OUT - PART  # 768
E = 16

NCORES = 8
BPC = B // NCORES        # batches per core = 4
TOK = BPC * N            # tokens per core  = 4096
TT = 512                 # token tile
NTILES = TOK // TT       # 8  (2 tiles per batch)
P = 128
KC = DIM // P            # 8  fc1 contraction chunks
HC = HID // P            # 32 hidden chunks
OC = OUT // P            # 8  output chunks (6 shared + 2 expert)
SC = SHARED // P         # 6

_CACHE: dict = {}


def _build_program():
    import concourse.tile as tile
    from concourse import bacc, mybir

    bf16 = mybir.dt.bfloat16
    f32 = mybir.dt.float32
    GELU = mybir.ActivationFunctionType.Gelu
    IDENT = mybir.ActivationFunctionType.Identity

    nc = bacc.Bacc()
    xT_d = nc.declare_dram_parameter("xT", [DIM, TOK], bf16, isOutput=False)
    w1T_d = nc.declare_dram_parameter("w1T", [DIM, HID], bf16, isOutput=False)
    b1T_d = nc.declare_dram_parameter("b1T", [P, HC], f32, isOutput=False)
    w2T_d = nc.declare_dram_parameter("w2T", [HID, SHARED], bf16, isOutput=False)
    weT_d = nc.declare_dram_parameter("weT", [BPC, HID, PART], bf16, isOutput=False)
    b2T_d = nc.declare_dram_parameter("b2T", [P, BPC * OC], f32, isOutput=False)
    outT_d = nc.declare_dram_parameter("outT", [OUT, TOK], f32, isOutput=True)

    with tile.TileContext(nc) as tc:
        with (
            tc.tile_pool(name="wsb", bufs=1) as wsb,      # resident weights
            tc.tile_pool(name="wesb", bufs=1) as wesb,    # expert weights (per batch)
            tc.tile_pool(name="bsb", bufs=1) as bsb,      # biases
            tc.tile_pool(name="xsb", bufs=2) as xsb,      # x tiles, double buffered
            tc.tile_pool(name="hsb", bufs=1) as hsb,      # gelu output chunks
            tc.tile_pool(name="osb", bufs=4) as osb,      # out staging
            tc.tile_pool(name="hps", bufs=2, space="PSUM") as hps,
            tc.tile_pool(name="ops", bufs=6, space="PSUM") as ops,
        ):
            # ---- load order matters: the first fc1 chain needs x tile 0 +
            # w1 only; w2 / expert weights aren't read until the PE is ~55us
            # in, so they load behind the critical path.
            b1_t = bsb.tile([P, HC], f32, tag="b1")
            nc.gpsimd.dma_start(b1_t[:], b1T_d[:, :])
            b2_t = bsb.tile([P, BPC * OC], f32, tag="b2")
            nc.gpsimd.dma_start(b2_t[:], b2T_d[:, :])

            # PE warmup: dummy matmuls on an uninitialized scratch tile keep
            # the PE busy from the preamble until the first x/w1 bytes land,
            # so the HAM clock gate starts ramping toward 8/8 immediately.
            # The results are never read.
            scr = bsb.tile([P, TT], bf16, tag="scr")
            nc.vector.memset(scr[:], 0.0)
            for _ in range(20):
                wp = hps.tile([P, TT], f32, tag="hps", name="warm")
                nc.tensor.matmul(wp[:], scr[:, 0:P], scr[:], start=True, stop=True)

            def load_x(ti, engine=None):
                t0 = ti * TT
                tiles = []
                for kc in range(KC):
                    t = xsb.tile([P, TT], bf16, tag=f"x_{kc}")
                    (engine or nc.sync).dma_start(
                        t[:], xT_d[kc * P:(kc + 1) * P, t0:t0 + TT]
                    )
                    tiles.append(t)
                return tiles

            def load_we(b, we_t):
                for hc in range(HC):
                    t = wesb.tile([P, PART], bf16, tag=f"we_{hc}")
                    nc.sync.dma_start(t[:], weT_d[b, hc * P:(hc + 1) * P, :])
                    we_t[hc] = t

            x_pending = load_x(0)

            # w1 loads in 512-wide column slices, earliest-needed first: the
            # fc1 chain for hid chunk hc only reads w1 columns
            # [hc*128,(hc+1)*128), so the PE can start once slice 0 (hid
            # chunks 0..3) has landed.
            w1_t = [
                wsb.tile([P, HID], bf16, tag=f"w1_{kc}", name=f"w1_{kc}")
                for kc in range(KC)
            ]
            HS = 512
            for q in range(HID // HS):
                for kc in range(KC):
                    nc.sync.dma_start(
                        w1_t[kc][:, q * HS:(q + 1) * HS],
                        w1T_d[kc * P:(kc + 1) * P, q * HS:(q + 1) * HS],
                    )

            w2_t = []
            for hc in range(HC):
                t = wsb.tile([P, SHARED], bf16, tag=f"w2_{hc}")
                nc.sync.dma_start(t[:], w2T_d[hc * P:(hc + 1) * P, :])
                w2_t.append(t)

            we_t = [None] * HC  # current batch's expert weight chunks
            load_we(0, we_t)

            for ti in range(NTILES):
                b = ti // (NTILES // BPC)
                t0 = ti * TT
                if ti % (NTILES // BPC) == 0 and ti > 0:
                    load_we(b, we_t)

                x_t = x_pending
                if ti + 1 < NTILES:
                    x_pending = load_x(ti + 1)

                # fc1 + erf-gelu: h^T[hid, tok] per 128-row chunk
                h_t = []
                for hc in range(HC):
                    acc = hps.tile([P, TT], f32, tag="hps")
                    for kc in range(KC):
                        nc.tensor.matmul(
                            acc[:],
                            w1_t[kc][:, hc * P:(hc + 1) * P],
                            x_t[kc][:],
                            start=(kc == 0),
                            stop=(kc == KC - 1),
                        )
                    h = hsb.tile([P, TT], bf16, tag=f"h_{hc}")
                    nc.scalar.activation(
                        h[:], acc[:], GELU, bias=b1_t[:, hc:hc + 1], scale=1.0
                    )
                    h_t.append(h)

                # fc2 (shared) + expert projection: out^T[out, tok]
                for oc in range(OC):
                    acc = ops.tile([P, TT], f32, tag="ops")
                    for hc in range(HC):
                        if oc < SC:
                            w = w2_t[hc][:, oc * P:(oc + 1) * P]
                        else:
                            w = we_t[hc][:, (oc - SC) * P:(oc - SC + 1) * P]
                        nc.tensor.matmul(
                            acc[:], w, h_t[hc][:],
                            start=(hc == 0), stop=(hc == HC - 1),
                        )
                    o = osb.tile([P, TT], f32, tag="o")
                    nc.scalar.activation(
                        o[:], acc[:], IDENT,
                        bias=b2_t[:, b * OC + oc:b * OC + oc + 1], scale=1.0,
                    )
                    nc.sync.dma_start(outT_d[oc * P:(oc + 1) * P, t0:t0 + TT], o[:])

    nc.finalize()
    return nc


def _get_program():
    if "nc" not in _CACHE:
        _CACHE["nc"] = _build_program()
    return _CACHE["nc"]


def _prep_in_maps(x, indices, fc1_w, fc1_b, fc2_w, fc2_b, experts_w, experts_b):
    bf16 = ml_dtypes.bfloat16
    x = np.asarray(x, dtype=np.float32)
    indices = np.asarray(indices).astype(np.int64)
    fc1_w = np.asarray(fc1_w, dtype=np.float32)
    fc1_b = np.asarray(fc1_b, dtype=np.float32)
    fc2_w = np.asarray(fc2_w, dtype=np.float32)
    fc2_b = np.asarray(fc2_b, dtype=np.float32)
    experts_w = np.asarray(experts_w, dtype=np.float32)
    experts_b = np.asarray(experts_b, dtype=np.float32)

    w1T = fc1_w.T.astype(bf16)                       # [DIM, HID]
    b1T = np.ascontiguousarray(fc1_b.reshape(HC, P).T)   # [P, HC]
    w2T = fc2_w.T.astype(bf16)                       # [HID, SHARED]

    in_maps = []
    for c in range(NCORES):
        idx = indices[c * BPC:(c + 1) * BPC]         # [BPC]
        xs = x[c * BPC:(c + 1) * BPC]                # [BPC, N, DIM]
        xT = xs.reshape(TOK, DIM).T.astype(bf16)     # [DIM, TOK]
        weT = experts_w[idx].transpose(0, 2, 1).astype(bf16)  # [BPC, HID, PART]
        b2 = np.concatenate(
            [np.broadcast_to(fc2_b, (BPC, SHARED)), experts_b[idx]], axis=1
        )                                            # [BPC, OUT]
        b2T = np.ascontiguousarray(
            b2.reshape(BPC, OC, P).transpose(2, 0, 1).reshape(P, BPC * OC)
        ).astype(np.float32)                         # [P, BPC*OC]
        in_maps.append({
            "xT": xT, "w1T": w1T, "b1T": b1T, "w2T": w2T,
            "weT": weT, "b2T": b2T,
        })
    return in_maps


def _assemble_output(results):
    out = np.empty((B, N, OUT), dtype=np.float32)
    for c in range(NCORES):
        outT = results[c]["outT"]                    # [OUT, TOK]
        out[c * BPC:(c + 1) * BPC] = outT.T.reshape(BPC, N, OUT)
    return out


def run_on_device(inputs: dict, trace: bool = False):
    """Run the SPMD program; returns (full_output, BassKernelResults)."""
    from concourse.bass_utils import run_bass_kernel_spmd

    nc = _get_program()
    in_maps = _prep_in_maps(**inputs)
    res = run_bass_kernel_spmd(nc, in_maps, list(range(NCORES)), trace=trace)
    return _assemble_output(res.results), res


def kernel(**inputs) -> np.ndarray:
    out, _ = run_on_device(inputs, trace=False)
    return out


# revision 13
# speedup vs baseline: 1.0037x; 1.0015x over previous
"""Trainium2 Bass kernel for MoE feed-forward (nn_MoEFeedForward_12292196401617).

Reference computation (per batch b of 32, N=1024 tokens, DIM=1024):
    h      = gelu_erf(x @ fc1_w.T + fc1_b)                  # [B,N,HID=4096]
    shared = h @ fc2_w.T + fc2_b                            # [B,N,768]
    expert = h @ experts_w[idx[b]].T + experts_b[idx[b]]    # [B,N,256]
    out    = concat([shared, expert], -1)                   # [B,N,1024]

Strategy: data-parallel over batch across 8 NeuronCores (4 batches/core).
The expert gather is resolved on the host (indices are per-batch), so the
device program is pure dense matmul. Everything is laid out
feature-on-partitions / tokens-on-free-dim, so all host->device tensors are
pre-transposed on the host. Weights are cast to bf16 (PE runs bf16 at 1
cycle/row vs 4 for fp32) and stay resident in SBUF; accumulation in fp32
PSUM; gelu runs on the scalar engine (erf-exact Gelu) fused with the fc1
bias add; the fc2/expert bias add is fused into the PSUM->SBUF eviction.
"""

import sys

sys.path.insert(0, "/opt/trn_rl_repo")

import numpy as np
import ml_dtypes

B, N, DIM = 32, 1024, 1024
HID = 4096
PART = 256
OUT = 1024
SHARED = OUT - PART  # 768
E = 16

NCORES = 8
BPC = B // NCORES        # batches per core = 4
TOK = BPC * N            # tokens per core  = 4096
TT = 512                 # token tile
NTILES = TOK // TT       # 8  (2 tiles per batch)
P = 128
KC = DIM // P            # 8  fc1 contraction chunks
HC = HID // P            # 32 hidden chunks
OC = OUT // P            # 8  output chunks (6 shared + 2 expert)
SC = SHARED // P         # 6

_CACHE: dict = {}


def _build_program():
    import concourse.tile as tile
    from concourse import bacc, mybir

    bf16 = mybir.dt.float16  # fp16: same PE rate as bf16, 8x the mantissa
    f32 = mybir.dt.float32
    GELU = mybir.ActivationFunctionType.Gelu
    IDENT = mybir.ActivationFunctionType.Identity

    nc = bacc.Bacc()
    xT_d = nc.declare_dram_parameter("xT", [DIM, TOK], bf16, isOutput=False)
    w1T_d = nc.declare_dram_parameter("w1T", [DIM, HID], bf16, isOutput=False)
    b1T_d = nc.declare_dram_parameter("b1T", [P, HC], f32, isOutput=False)
    w2T_d = nc.declare_dram_parameter("w2T", [HID, SHARED], bf16, isOutput=False)
    weT_d = nc.declare_dram_parameter("weT", [BPC, HID, PART], bf16, isOutput=False)
    b2T_d = nc.declare_dram_parameter("b2T", [P, BPC * OC], f32, isOutput=False)
    outT_d = nc.declare_dram_parameter("outT", [OUT, TOK], f32, isOutput=True)

    with tile.TileContext(nc) as tc:
        with (
            tc.tile_pool(name="wsb", bufs=1) as wsb,      # resident weights
            tc.tile_pool(name="wesb", bufs=1) as wesb,    # expert weights (per batch)
            tc.tile_pool(name="bsb", bufs=1) as bsb,      # biases
            tc.tile_pool(name="xsb", bufs=2) as xsb,      # x tiles, double buffered
            tc.tile_pool(name="hsb", bufs=1) as hsb,      # gelu output chunks
            tc.tile_pool(name="osb", bufs=4) as osb,      # out staging
            tc.tile_pool(name="hps", bufs=2, space="PSUM") as hps,
            tc.tile_pool(name="ops", bufs=6, space="PSUM") as ops,
        ):
            # ---- load order matters: the first fc1 chain needs x tile 0 +
            # w1 only; w2 / expert weights aren't read until the PE is ~55us
            # in, so they load behind the critical path.
            b1_t = bsb.tile([P, HC], f32, tag="b1")
            nc.gpsimd.dma_start(b1_t[:], b1T_d[:, :])
            b2_t = bsb.tile([P, BPC * OC], f32, tag="b2")
            nc.gpsimd.dma_start(b2_t[:], b2T_d[:, :])

            # PE warmup: dummy matmuls on an uninitialized scratch tile keep
            # the PE busy from the preamble until the first x/w1 bytes land,
            # so the HAM clock gate starts ramping toward 8/8 immediately.
            # The results are never read.
            scr = bsb.tile([P, TT], bf16, tag="scr")
            nc.vector.memset(scr[:], 0.0)
            for _ in range(20):
                wp = hps.tile([P, TT], f32, tag="hps", name="warm")
                nc.tensor.matmul(wp[:], scr[:, 0:P], scr[:], start=True, stop=True)

            def load_x(ti, engine=None):
                t0 = ti * TT
                tiles = []
                for kc in range(KC):
                    t = xsb.tile([P, TT], bf16, tag=f"x_{kc}")
                    (engine or nc.sync).dma_start(
                        t[:], xT_d[kc * P:(kc + 1) * P, t0:t0 + TT]
                    )
                    tiles.append(t)
                return tiles

            def load_we(b, we_t):
                for hc in range(HC):
                    t = wesb.tile([P, PART], bf16, tag=f"we_{hc}")
                    nc.sync.dma_start(t[:], weT_d[b, hc * P:(hc + 1) * P, :])
                    we_t[hc] = t

            x_pending = load_x(0)

            # w1 loads in 512-wide column slices, earliest-needed first: the
            # fc1 chain for hid chunk hc only reads w1 columns
            # [hc*128,(hc+1)*128), so the PE can start once slice 0 (hid
            # chunks 0..3) has landed.
            w1_t = [
                wsb.tile([P, HID], bf16, tag=f"w1_{kc}", name=f"w1_{kc}")
                for kc in range(KC)
            ]
            HS = 512
            for q in range(HID // HS):
                for kc in range(KC):
                    nc.sync.dma_start(
                        w1_t[kc][:, q * HS:(q + 1) * HS],
                        w1T_d[kc * P:(kc + 1) * P, q * HS:(q + 1) * HS],
                    )

            w2_t = []
            for hc in range(HC):
                t = wsb.tile([P, SHARED], bf16, tag=f"w2_{hc}")
                nc.sync.dma_start(t[:], w2T_d[hc * P:(hc + 1) * P, :])
                w2_t.append(t)

            we_t = [None] * HC  # current batch's expert weight chunks
            load_we(0, we_t)

            for ti in range(NTILES):
                b = ti // (NTILES // BPC)
                t0 = ti * TT
                if ti % (NTILES // BPC) == 0 and ti > 0:
                    load_we(b, we_t)

                x_t = x_pending
                if ti + 1 < NTILES:
                    x_pending = load_x(ti + 1)

                # fc1 + erf-gelu: h^T[hid, tok] per 128-row chunk
                h_t = []
                for hc in range(HC):
                    acc = hps.tile([P, TT], f32, tag="hps")
                    for kc in range(KC):
                        nc.tensor.matmul(
                            acc[:],
                            w1_t[kc][:, hc * P:(hc + 1) * P],
                            x_t[kc][:],
                            start=(kc == 0),
                            stop=(kc == KC - 1),
                        )
                    h = hsb.tile([P, TT], bf16, tag=f"h_{hc}")
                    nc.scalar.activation(
                        h[:], acc[:], GELU, bias=b1_t[:, hc:hc + 1], scale=1.0
                    )
                    h_t.append(h)

                # fc2 (shared) + expert projection: out^T[out, tok]
                for oc in range(OC):
                    acc = ops.tile([P, TT], f32, tag="ops")
                    for hc in range(HC):
                        if oc < SC:
                            w = w2_t[hc][:, oc * P:(oc + 1) * P]
                        else:
                            w = we_t[hc][:, (oc - SC) * P:(oc - SC + 1) * P]
                        nc.tensor.matmul(
                            acc[:], w, h_t[hc][:],
                            start=(hc == 0), stop=(hc == HC - 1),
                        )
                    o = osb.tile([P, TT], f32, tag="o")
                    nc.scalar.activation(
                        o[:], acc[:], IDENT,
                        bias=b2_t[:, b * OC + oc:b * OC + oc + 1], scale=1.0,
                    )
                    nc.sync.dma_start(outT_d[oc * P:(oc + 1) * P, t0:t0 + TT], o[:])

    nc.finalize()
    return nc


def _get_program():
    if "nc" not in _CACHE:
        _CACHE["nc"] = _build_program()
    return _CACHE["nc"]


def _prep_in_maps(x, indices, fc1_w, fc1_b, fc2_w, fc2_b, experts_w, experts_b):
    bf16 = np.float16
    x = np.asarray(x, dtype=np.float32)
    indices = np.asarray(indices).astype(np.int64)
    fc1_w = np.asarray(fc1_w, dtype=np.float32)
    fc1_b = np.asarray(fc1_b, dtype=np.float32)
    fc2_w = np.asarray(fc2_w, dtype=np.float32)
    fc2_b = np.asarray(fc2_b, dtype=np.float32)
    experts_w = np.asarray(experts_w, dtype=np.float32)
    experts_b = np.asarray(experts_b, dtype=np.float32)

    w1T = fc1_w.T.astype(bf16)                       # [DIM, HID]
    b1T = np.ascontiguousarray(fc1_b.reshape(HC, P).T)   # [P, HC]
    w2T = fc2_w.T.astype(bf16)                       # [HID, SHARED]

    in_maps = []
    for c in range(NCORES):
        idx = indices[c * BPC:(c + 1) * BPC]         # [BPC]
        xs = x[c * BPC:(c + 1) * BPC]                # [BPC, N, DIM]
        xT = xs.reshape(TOK, DIM).T.astype(bf16)     # [DIM, TOK]
        weT = experts_w[idx].transpose(0, 2, 1).astype(bf16)  # [BPC, HID, PART]
        b2 = np.concatenate(
            [np.broadcast_to(fc2_b, (BPC, SHARED)), experts_b[idx]], axis=1
        )                                            # [BPC, OUT]
        b2T = np.ascontiguousarray(
            b2.reshape(BPC, OC, P).transpose(2, 0, 1).reshape(P, BPC * OC)
        ).astype(np.float32)                         # [P, BPC*OC]
        in_maps.append({
            "xT": xT, "w1T": w1T, "b1T": b1T, "w2T": w2T,
            "weT": weT, "b2T": b2T,
        })
    return in_maps


def _assemble_output(results):
    out = np.empty((B, N, OUT), dtype=np.float32)
    for c in range(NCORES):
        outT = results[c]["outT"]                    # [OUT, TOK]
        out[c * BPC:(c + 1) * BPC] = outT.T.reshape(BPC, N, OUT)
    return out


def run_on_device(inputs: dict, trace: bool = False):
    """Run the SPMD program; returns (full_output, BassKernelResults)."""
    from concourse.bass_utils import run_bass_kernel_spmd

    nc = _get_program()
    in_maps = _prep_in_maps(**inputs)
    res = run_bass_kernel_spmd(nc, in_maps, list(range(NCORES)), trace=trace)
    return _assemble_output(res.results), res


def kernel(**inputs) -> np.ndarray:
    out, _ = run_on_device(inputs, trace=False)
    return out
